# revision 9
# baseline (speedup 1.0000x reference)
"""Trainium2 Bass kernel for a Blenderbot decoder layer (prefill).

Sharding: 8-way tensor parallel over attention heads (4 heads / core) and
FFN columns, Megatron-style, with sequence-parallel residual/LayerNorm
sections between the blocks (ReduceScatter -> token-sharded residual+LN ->
AllGather).  Host pre-shards, pre-folds LayerNorm gamma/beta and the
1/sqrt(hd) attention scale into the projection weights, pre-transposes the
encoder activations, and pre-casts weights to bf16.  Device computes all
matmuls in bf16 (fp32 accumulation in PSUM), LayerNorm statistics and
residuals in fp32.
"""

import math

import numpy as np
import ml_dtypes

B, S, D = 4, 128, 2560
H, HD = 32, 80
FFN = 10240
DEC = ENC = 256
EPS = 1e-5
NCORES = 8
T = B * S                 # 512 tokens
HPC = H // NCORES         # 4 heads per core
NQ = HPC * HD             # 320 head-dim columns per core
NQP = 384                 # NQ padded to a multiple of 128
FFNS = FFN // NCORES      # 1280 ffn columns per core
SH = T // NCORES          # 64-token shard per core
KT = D // 128             # 20 K tiles over D
KF = FFNS // 128          # 10 tiles over ffn shard
NTOK = T // 128           # 4 token tiles
NCH = D // 512            # 5 moving-dim chunks of the output projections
KO = NQP // 128           # 3 K tiles over the padded head block
BF16 = ml_dtypes.bfloat16

_STATE = {}


def _build():
    import concourse.bacc as bacc
    import concourse.mybir as mybir
    import concourse.tile as tile

    dt = mybir.dt
    F32, BF = dt.float32, dt.bfloat16
    AF = mybir.ActivationFunctionType
    ALU = mybir.AluOpType
    RG = [list(range(NCORES))]

    nc = bacc.Bacc("TRN2", target_bir_lowering=False, debug=False,
                   num_devices=NCORES)

    def din(name, shape, dtype):
        return nc.dram_tensor(name, list(shape), dtype, kind="ExternalInput").ap()

    def dout(name, shape, dtype):
        return nc.dram_tensor(name, list(shape), dtype, kind="ExternalOutput").ap()

    x_in = din("x", (T, D), F32)
    xsb_in = din("xsb", (SH, D), F32)        # x shard with sa_bo folded in
    xaT_in = din("xaT", (D, T), BF)
    w_ins = {}
    for nm, shp in (("wq1", (D, NQ)), ("wk1", (D, NQ)), ("wv1", (D, NQ)),
                    ("wq2", (D, NQ)), ("wk2", (D, NQ)), ("wv2", (D, NQ)),
                    ("wo1", (NCH * KO * 128, 512)),   # (nch, kb) blocks
                    ("wo2", (NCH * KO * 128, 512)),
                    ("fc1w", (KF * KT * 128, 128)),   # (m, k) blocks
                    ("fc2w", (NCH * KF * 128, 512))):  # (nch, k) blocks
        w_ins[nm] = din(nm, shp, BF)
    qb1_in = din("qb1", (128, HPC), F32)     # col h = head-h bias (80 rows used)
    kb1_in = din("kb1", (128, HPC), F32)
    qb2_in = din("qb2", (128, HPC), F32)
    kb2_in = din("kb2", (128, HPC), F32)
    vb1_in = din("vb1", (1, NQ), BF)
    vb2_in = din("vb2", (1, NQ), BF)
    fc1b_in = din("fc1b", (128, KF), F32)
    maskT_in = din("maskT", (128, T), BF)    # [k, b*128+q] 0/1 self-attn mask
    crossb_in = din("crossb", (128, B), F32)  # additive bias per (k, b)
    bo2t_in = din("bo2t", (SH, D), BF)
    fb2t_in = din("fb2t", (SH, D), BF)

    kT1_out = dout("kT1", (NQ, T), BF)
    vt1_out = dout("vt1", (T, NQ), BF)
    kT2_out = dout("kT2", (NQ, T), BF)
    vt2_out = dout("vt2", (T, NQ), BF)
    x_out = dout("xout", (SH, D), F32)

    with tile.TileContext(nc) as tc:
        with tc.tile_pool(name="dram", bufs=1, space="DRAM") as dpool, \
             tc.tile_pool(name="psum", bufs=8, space="PSUM") as pspool, \
             tc.tile_pool(name="sb", bufs=1) as sb:

            counter = [0]

            def alloc(tag, shape, dtype, bufs):
                counter[0] += 1
                return sb.tile(shape, dtype, name=f"{tag}{counter[0]}",
                               tag=tag, bufs=bufs)

            def psum(shape=(128, 512)):
                return pspool.tile(list(shape), F32, name="ps", tag="ps")

            # ---------------- DRAM bounce buffers for collectives ----------
            rs1i = dpool.tile([T, D], BF, name="rs1i")
            rs1o = dpool.tile([SH, D], BF, name="rs1o")
            ag1i = dpool.tile([SH, D], BF, name="ag1i")
            ag1o = dpool.tile([T, D], BF, name="ag1o", addr_space="Shared")
            rs2i = dpool.tile([T, D], BF, name="rs2i")
            rs2o = dpool.tile([SH, D], BF, name="rs2o")
            ag2i = dpool.tile([SH, D], BF, name="ag2i")
            ag2o = dpool.tile([T, D], BF, name="ag2o", addr_space="Shared")
            rs3i = dpool.tile([T, D], BF, name="rs3i")
            rs3o = dpool.tile([SH, D], BF, name="rs3o")

            # ---------------- small constants (live whole kernel) ---------
            qb1_sb = sb.tile([128, HPC], F32, name="qb1_sb")
            kb1_sb = sb.tile([128, HPC], F32, name="kb1_sb")
            qb2_sb = sb.tile([128, HPC], F32, name="qb2_sb")
            kb2_sb = sb.tile([128, HPC], F32, name="kb2_sb")
            vb1_sb = sb.tile([1, NQ], BF, name="vb1_sb")
            vb2_sb = sb.tile([1, NQ], BF, name="vb2_sb")
            fc1b_sb = sb.tile([128, KF], F32, name="fc1b_sb")
            maskT_sb = sb.tile([128, T], BF, name="maskT_sb")
            crossb_sb = sb.tile([128, B], F32, name="crossb_sb")
            onesc_sb = sb.tile([128, 1], BF, name="onesc_sb")   # ones column
            onesr_sb = sb.tile([1, 128], BF, name="onesr_sb")   # ones row
            eps_sb = sb.tile([128, 1], F32, name="eps_sb")
            for t_, s_ in ((qb1_sb, qb1_in), (kb1_sb, kb1_in), (qb2_sb, qb2_in),
                           (kb2_sb, kb2_in), (vb1_sb, vb1_in), (vb2_sb, vb2_in),
                           (fc1b_sb, fc1b_in), (maskT_sb, maskT_in),
                           (crossb_sb, crossb_in)):
                nc.sync.dma_start(t_[:], s_[:])
            nc.vector.memset(onesc_sb[:], 1.0)
            nc.vector.memset(onesr_sb[:], 1.0)
            nc.vector.memset(eps_sb[:], EPS)

            # residual accumulator (in-place through the whole chain)
            xres = alloc("xres", [SH, D], F32, 1)
            nc.sync.dma_start(xres[:], xsb_in[:])

            # ---------------- loaders -------------------
            def load_wcol(nm):
                # [D, NQ] -> KT tiles of [128, NQ]
                ts = []
                for k in range(KT):
                    w = alloc("w320", [128, NQ], BF, 60)
                    nc.sync.dma_start(w[:], w_ins[nm][128 * k:128 * (k + 1), :])
                    ts.append(w)
                return ts

            def load_block(nm, tag, i, bufs):
                w = alloc(tag, [128, 512], BF, bufs)
                nc.sync.dma_start(w[:], w_ins[nm][128 * i:128 * (i + 1), :])
                return w

            # ---------------- LayerNorm helper ----------------
            def layernorm(dst_bf, src_f32, P):
                # src [P, D] fp32 -> dst [P, D] bf16 normalized (no gamma/beta)
                bnt = alloc("ln_bnt", [P, 5 * 6], F32, 4)
                mv = alloc("ln_mv", [P, 2], F32, 4)
                istd = alloc("ln_istd", [P, 1], F32, 4)
                nmu = alloc("ln_nmu", [P, 1], F32, 4)
                for c in range(5):
                    nc.vector.bn_stats(bnt[:, 6 * c:6 * (c + 1)],
                                       src_f32[:, 512 * c:512 * (c + 1)])
                nc.vector.bn_aggr(mv[:], bnt[:])
                nc.scalar.activation(istd[:], mv[:, 1:2], AF.Sqrt,
                                     bias=eps_sb[0:P, :])
                nc.vector.reciprocal(istd[:], istd[:])
                nc.vector.tensor_mul(nmu[:], mv[:, 0:1], istd[:])
                nc.vector.tensor_scalar_mul(nmu[:], nmu[:], -1.0)
                nc.scalar.activation(dst_bf, src_f32, AF.Identity,
                                     bias=nmu[:], scale=istd[:])

            # ---------------- projections ----------------
            def proj_qk(wtiles, bias_sb, hT, name):
                # per-head feature-major tiles [80, T]
                out_tiles = []
                for h in range(HPC):
                    ps = psum()
                    for k in range(KT):
                        nc.tensor.matmul(
                            ps[0:80, :], wtiles[k][:, 80 * h:80 * h + 80],
                            hT[k][:], start=(k == 0), stop=(k == KT - 1))
                    o = alloc("qkT", [80, T], BF, 12)
                    nc.scalar.activation(o[:], ps[0:80, :], AF.Identity,
                                         bias=bias_sb[0:80, h:h + 1])
                    out_tiles.append(o)
                return out_tiles

            def proj_v(wtiles, bias_sb, hT, name):
                # token-major [128, NQ] tiles; bias added as a rank-1 matmul
                out_tiles = []
                for mt in range(NTOK):
                    ps = psum((128, NQ))
                    nc.tensor.matmul(ps[:], onesr_sb[:], bias_sb[:],
                                     start=True, stop=False)
                    for k in range(KT):
                        nc.tensor.matmul(
                            ps[:], hT[k][:, 128 * mt:128 * (mt + 1)],
                            wtiles[k][:], start=False, stop=(k == KT - 1))
                    o = alloc("vt", [128, NQ], BF, 8)
                    nc.scalar.copy(o[:], ps[:])
                    out_tiles.append(o)
                return out_tiles

            # ---------------- attention ----------------
            def attention(qT, kT, vtok, cross, name):
                # token-major o_cat [128, NQP] per batch (per-partition 1/Z),
                # then xbar-transposed into packed feature-major tiles.
                ocatT = [alloc("ocatT", [128, T], BF, 6) for _ in range(KO)]
                for b in range(B):
                    cs = slice(128 * b, 128 * (b + 1))
                    oc = alloc("at_oc", [128, NQP], BF, 2)
                    nc.vector.memset(oc[:, NQ:NQP], 0.0)
                    for h in range(HPC):
                        sps = psum((128, 128))
                        nc.tensor.matmul(sps[:], kT[h][:, cs], qT[h][:, cs],
                                         start=True, stop=True)
                        p = alloc("at_p", [128, 128], BF, 4)
                        if cross:
                            nc.scalar.activation(p[:], sps[:], AF.Exp,
                                                 bias=crossb_sb[:, b:b + 1])
                        else:
                            nc.scalar.activation(p[:], sps[:], AF.Exp)
                            nc.vector.tensor_mul(p[:], p[:], maskT_sb[:, cs])
                        ops = psum((128, 128))
                        nc.tensor.matmul(ops[:, 0:80], p[:],
                                         vtok[b][:, 80 * h:80 * h + 80],
                                         start=True, stop=True)
                        nc.tensor.matmul(ops[:, 80:81], p[:], onesc_sb[:],
                                         start=True, stop=True)
                        zinv = alloc("at_zi", [128, 1], F32, 4)
                        nc.vector.reciprocal(zinv[:], ops[:, 80:81])
                        nc.scalar.activation(oc[:, 80 * h:80 * h + 80],
                                             ops[:, 0:80], AF.Copy,
                                             scale=zinv[:])
                    for c in range(KO):
                        nc.sync.dma_start(ocatT[c][:, cs],
                                          oc[:, 128 * c:128 * (c + 1)],
                                          transpose=True)
                return ocatT

            # ---------------- output projection ----------------
            def out_proj(ocatT, wo_nm, dst_dram):
                for nch in range(NCH):
                    wos = [load_block(wo_nm, "wo", nch * KO + k, 12)
                           for k in range(KO)]
                    for mt in range(NTOK):
                        ps = psum()
                        for k in range(KO):
                            nc.tensor.matmul(
                                ps[:], ocatT[k][:, 128 * mt:128 * (mt + 1)],
                                wos[k][:], start=(k == 0), stop=(k == KO - 1))
                        stage = alloc("stage", [128, 512], BF, 3)
                        nc.scalar.copy(stage[:], ps[:])
                        nc.sync.dma_start(
                            dst_dram[128 * mt:128 * (mt + 1),
                                     512 * nch:512 * (nch + 1)], stage[:])

            # ---------------- sharded residual + LN ----------------
            def resid_ln(rs_out_dram, bias_in, ag_in_dram, name, last=False):
                rssb = alloc("res16", [SH, D], BF, 2)
                nc.sync.dma_start(rssb[:], rs_out_dram[:])
                nc.vector.tensor_add(xres[:], xres[:], rssb[:])
                if bias_in is not None:
                    bt = alloc("biast", [SH, D], BF, 1)
                    nc.sync.dma_start(bt[:], bias_in[:])
                    nc.vector.tensor_add(xres[:], xres[:], bt[:])
                if last:
                    nc.sync.dma_start(x_out[:], xres[:])
                    return
                hsh = alloc("res16", [SH, D], BF, 2)
                layernorm(hsh[:], xres[:], SH)
                nc.sync.dma_start(ag_in_dram[:], hsh[:])

            def collective(kind, op, i, o):
                nc.gpsimd.collective_compute(kind, op, replica_groups=RG,
                                             ins=[i.opt()], outs=[o.opt()])

            def load_T(src_dram, name):
                ts = []
                for k in range(KT):
                    t_ = alloc("actT", [128, T], BF, 40)
                    nc.sync.dma_start(t_[:],
                                      src_dram[:, 128 * k:128 * (k + 1)],
                                      transpose=True)
                    ts.append(t_)
                return ts

            # =================== phase 0: LN1 + transpose ==============
            wq1 = load_wcol("wq1")
            wk1 = load_wcol("wk1")
            wv1 = load_wcol("wv1")
            xaT = []
            for k in range(KT):
                t_ = alloc("actT", [128, T], BF, 40)
                nc.sync.dma_start(t_[:], xaT_in[128 * k:128 * (k + 1), :])
                xaT.append(t_)
            h1T = [alloc("actT", [128, T], BF, 40) for _ in range(KT)]
            for mt in range(NTOK):
                xt = alloc("xt", [128, D], F32, 1)
                nc.sync.dma_start(xt[:], x_in[128 * mt:128 * (mt + 1), :])
                h1t = alloc("h1t", [128, D], BF, 1)
                layernorm(h1t[:], xt[:], 128)
                for k in range(KT):
                    nc.sync.dma_start(
                        h1T[k][:, 128 * mt:128 * (mt + 1)],
                        h1t[:, 128 * k:128 * (k + 1)], transpose=True)

            # =================== self attention ===================
            qT1 = proj_qk(wq1, qb1_sb, h1T, "qT1")
            kT1 = proj_qk(wk1, kb1_sb, h1T, "kT1")
            vt1 = proj_v(wv1, vb1_sb, h1T, "vt1")
            for h in range(HPC):
                nc.sync.dma_start(kT1_out[80 * h:80 * h + 80, :], kT1[h][:])
            for mt in range(NTOK):
                nc.sync.dma_start(vt1_out[128 * mt:128 * (mt + 1), :],
                                  vt1[mt][:])
            oc1 = attention(qT1, kT1, vt1, cross=False, name="a1")
            out_proj(oc1, "wo1", rs1i)
            collective("ReduceScatter", ALU.add, rs1i, rs1o)

            # cross K/V (independent of the self-attention result)
            wk2 = load_wcol("wk2")
            wv2 = load_wcol("wv2")
            kT2 = proj_qk(wk2, kb2_sb, xaT, "kT2")
            vt2 = proj_v(wv2, vb2_sb, xaT, "vt2")
            for h in range(HPC):
                nc.sync.dma_start(kT2_out[80 * h:80 * h + 80, :], kT2[h][:])
            for mt in range(NTOK):
                nc.sync.dma_start(vt2_out[128 * mt:128 * (mt + 1), :],
                                  vt2[mt][:])

            resid_ln(rs1o, None, ag1i, "r1")
            collective("AllGather", ALU.bypass, ag1i, ag1o)
            h2T = load_T(ag1o, "h2T")

            # =================== cross attention ===================
            wq2 = load_wcol("wq2")
            qT2 = proj_qk(wq2, qb2_sb, h2T, "qT2")
            oc2 = attention(qT2, kT2, vt2, cross=True, name="a2")
            out_proj(oc2, "wo2", rs2i)
            collective("ReduceScatter", ALU.add, rs2i, rs2o)
            resid_ln(rs2o, bo2t_in, ag2i, "r2")
            collective("AllGather", ALU.bypass, ag2i, ag2o)
            h3T = load_T(ag2o, "h3T")

            # =================== MLP ===================
            uT = []
            for m in range(KF):
                ws = []
                for k in range(KT):
                    w = alloc("fc1w", [128, 128], BF, 40)
                    nc.sync.dma_start(
                        w[:], w_ins["fc1w"][128 * (m * KT + k):
                                            128 * (m * KT + k + 1), :])
                    ws.append(w)
                ps = psum()
                for k in range(KT):
                    nc.tensor.matmul(ps[:], ws[k][:], h3T[k][:],
                                     start=(k == 0), stop=(k == KT - 1))
                u = alloc("uT", [128, T], BF, 10)
                nc.scalar.activation(u[:], ps[:], AF.Gelu,
                                     bias=fc1b_sb[:, m:m + 1])
                uT.append(u)
            for nch in range(NCH):
                ws = [load_block("fc2w", "fc2w", nch * KF + k, 20)
                      for k in range(KF)]
                for mt in range(NTOK):
                    ps = psum()
                    for k in range(KF):
                        nc.tensor.matmul(
                            ps[:], uT[k][:, 128 * mt:128 * (mt + 1)],
                            ws[k][:], start=(k == 0), stop=(k == KF - 1))
                    stage = alloc("stage", [128, 512], BF, 3)
                    nc.scalar.copy(stage[:], ps[:])
                    nc.sync.dma_start(
                        rs3i[128 * mt:128 * (mt + 1),
                             512 * nch:512 * (nch + 1)], stage[:])
            collective("ReduceScatter", ALU.add, rs3i, rs3o)
            resid_ln(rs3o, fb2t_in, None, "r3", last=True)

    nc.compile()
    return nc


def _prep_inputs(inputs):
    f32 = np.float32
    x = np.asarray(inputs["x"], f32).reshape(T, D)
    xa = np.asarray(inputs["xa"], f32).reshape(T, D)
    mask = np.asarray(inputs["mask"])
    cross_mask = np.asarray(inputs["cross_mask"])
    g1 = np.asarray(inputs["ln1_g"], f32)
    b1 = np.asarray(inputs["ln1_b"], f32)
    g2 = np.asarray(inputs["ln2_g"], f32)
    b2 = np.asarray(inputs["ln2_b"], f32)
    g3 = np.asarray(inputs["ln3_g"], f32)
    b3 = np.asarray(inputs["ln3_b"], f32)
    scale = f32(1.0 / math.sqrt(HD))

    xaT = np.ascontiguousarray(xa.T).astype(BF16)

    def colshard(w, b, g, bfold, r, n, sc=1.0):
        w = np.asarray(w, f32)
        b = np.asarray(b, f32)
        cols = slice(n * r, n * (r + 1))
        weff = w[:, cols] * (g[:, None] * f32(sc))
        beff = (bfold @ w[:, cols] + b[cols]) * f32(sc)
        return np.ascontiguousarray(weff).astype(BF16), beff

    def headbias(beff):
        out = np.zeros((HPC, 128), f32)
        out[:, :HD] = beff.reshape(HPC, HD)
        return np.ascontiguousarray(out.T)

    def rowblocks(w, r, nrows):
        # [nrows, D] row shard -> padded [NQP, D] -> (nch, kb) blocks
        w = np.asarray(w, f32)[nrows * r:nrows * (r + 1), :]
        wp = np.zeros((NQP, D), f32)
        wp[:nrows] = w
        blk = wp.reshape(KO, 128, NCH, 512).transpose(2, 0, 1, 3)
        return np.ascontiguousarray(blk.reshape(NCH * KO * 128, 512)).astype(BF16)

    # self-attn mask -> multiplicative, transposed to [k, b*128+q]
    m01 = mask[:, 0, :, :S].astype(f32)                      # [B, q, k]
    maskT = np.ascontiguousarray(
        m01.transpose(2, 0, 1).reshape(S, B * S)).astype(BF16)
    cb = np.where(cross_mask[:, 0, 0, :S], f32(0), f32(-1e9))  # [B, k]
    crossb = np.ascontiguousarray(cb.T)                        # [k, B]

    ones_g = np.ones_like(g1)
    zb = np.zeros_like(b1)
    in_maps = []
    for r in range(NCORES):
        wq1, qb1 = colshard(inputs["sa_wq"], inputs["sa_bq"], g1, b1, r, NQ,
                            scale)
        wk1, kb1 = colshard(inputs["sa_wk"], inputs["sa_bk"], g1, b1, r, NQ)
        wv1, vb1 = colshard(inputs["sa_wv"], inputs["sa_bv"], g1, b1, r, NQ)
        wq2, qb2 = colshard(inputs["ca_wq"], inputs["ca_bq"], g2, b2, r, NQ,
                            scale)
        # cross K/V read raw xa: no LN fold
        wk2, kb2 = colshard(inputs["ca_wk"], inputs["ca_bk"], ones_g, zb, r, NQ)
        wv2, vb2 = colshard(inputs["ca_wv"], inputs["ca_bv"], ones_g, zb, r, NQ)
        fc1w, fc1b = colshard(inputs["fc1_w"], inputs["fc1_b"], g3, b3, r, FFNS)
        # fc1: (m, k) 128x128 blocks
        fc1blk = fc1w.astype(f32).reshape(KT, 128, KF, 128).transpose(
            2, 0, 1, 3).reshape(KF * KT * 128, 128)
        fc2 = np.asarray(inputs["fc2_w"], f32)[FFNS * r:FFNS * (r + 1), :]
        fc2blk = fc2.reshape(KF, 128, NCH, 512).transpose(
            2, 0, 1, 3).reshape(NCH * KF * 128, 512)
        xsb = x[SH * r:SH * (r + 1), :] + np.asarray(inputs["sa_bo"], f32)[None, :]
        in_maps.append({
            "x": x, "xsb": np.ascontiguousarray(xsb), "xaT": xaT,
            "wq1": wq1, "wk1": wk1, "wv1": wv1,
            "wq2": wq2, "wk2": wk2, "wv2": wv2,
            "wo1": rowblocks(inputs["sa_wo"], r, NQ),
            "wo2": rowblocks(inputs["ca_wo"], r, NQ),
            "fc1w": np.ascontiguousarray(fc1blk).astype(BF16),
            "fc2w": np.ascontiguousarray(fc2blk).astype(BF16),
            "qb1": headbias(qb1), "kb1": headbias(kb1),
            "qb2": headbias(qb2), "kb2": headbias(kb2),
            "vb1": vb1[None, :].astype(BF16), "vb2": vb2[None, :].astype(BF16),
            "fc1b": np.ascontiguousarray(fc1b.reshape(KF, 128).T),
            "maskT": maskT, "crossb": crossb,
            "bo2t": np.broadcast_to(np.asarray(inputs["ca_bo"], f32),
                                    (SH, D)).astype(BF16),
            "fb2t": np.broadcast_to(np.asarray(inputs["fc2_b"], f32),
                                    (SH, D)).astype(BF16),
        })
    return in_maps


def _gather(inputs, results):
    f32 = np.float32
    x_out = np.concatenate([results[r]["xout"] for r in range(NCORES)],
                           axis=0).reshape(B, S, D).astype(f32)

    def cache_fill(cache_in, key):
        out = np.array(cache_in, dtype=f32)
        for r in range(NCORES):
            arr = np.asarray(results[r][key], dtype=f32)
            if key.startswith("kT"):
                arr = arr.T                     # -> [T, NQ]
            blk = arr.reshape(B, S, HPC, HD).transpose(0, 2, 1, 3)
            out[:, HPC * r:HPC * (r + 1), :S, :] = blk
        return out

    sk = cache_fill(inputs["cache_sk"], "kT1")
    sv = cache_fill(inputs["cache_sv"], "vt1")
    ck = cache_fill(inputs["cache_ck"], "kT2")
    cv = cache_fill(inputs["cache_cv"], "vt2")
    return x_out, sk, sv, ck, cv


def _run(in_maps, trace=False):
    from concourse.bass_utils import run_bass_kernel_spmd
    return run_bass_kernel_spmd(_STATE["nc"], in_maps,
                                core_ids=list(range(NCORES)), trace=trace)


def kernel(**inputs):
    if "nc" not in _STATE:
        _STATE["nc"] = _build()
    in_maps = _prep_inputs(inputs)
    res = _run(in_maps)
    return _gather(inputs, res.results)


# revision 11
# speedup vs baseline: 1.3176x; 1.3176x over previous
"""Trainium2 Bass kernel for a Blenderbot decoder layer (prefill).

Sharding: 8-way tensor parallel over attention heads (4 heads / core) and
FFN columns, Megatron-style, with sequence-parallel residual/LayerNorm
sections between the blocks (ReduceScatter -> token-sharded residual+LN ->
AllGather).  Host pre-shards, pre-folds LayerNorm gamma/beta and the
1/sqrt(hd) attention scale into the projection weights, packs weights into
the exact SBUF tile layouts (one large contiguous DMA each), pre-transposes
the encoder activations, and pre-casts to bf16.  Device computes matmuls in
bf16 (fp32 PSUM accumulation); LayerNorm statistics and residuals in fp32.
All activation transposes run on the tensor engine (PE transpose mode) —
the xbar DMA-transpose path is serialized in hardware and too slow.
"""

import math

import numpy as np
import ml_dtypes

B, S, D = 4, 128, 2560
H, HD = 32, 80
FFN = 10240
DEC = ENC = 256
EPS = 1e-5
NCORES = 8
T = B * S                 # 512 tokens
HPC = H // NCORES         # 4 heads per core
NQ = HPC * HD             # 320 head-dim columns per core
NQP = 384                 # NQ padded to a multiple of 128
FFNS = FFN // NCORES      # 1280 ffn columns per core
SH = T // NCORES          # 64-token shard per core
KT = D // 128             # 20 K tiles over D
KF = FFNS // 128          # 10 tiles over ffn shard
NTOK = T // 128           # 4 token tiles
NCH = D // 512            # 5 moving-dim chunks of the output projections
KO = NQP // 128           # 3 K tiles over the padded head block
BF16 = ml_dtypes.bfloat16

_STATE = {}


def _build():
    import concourse.bacc as bacc
    import concourse.mybir as mybir
    import concourse.tile as tile

    dt = mybir.dt
    F32, BF = dt.float32, dt.bfloat16
    AF = mybir.ActivationFunctionType
    ALU = mybir.AluOpType
    RG = [list(range(NCORES))]

    nc = bacc.Bacc("TRN2", target_bir_lowering=False, debug=False,
                   num_devices=NCORES)

    def din(name, shape, dtype):
        return nc.dram_tensor(name, list(shape), dtype, kind="ExternalInput").ap()

    def dout(name, shape, dtype):
        return nc.dram_tensor(name, list(shape), dtype, kind="ExternalOutput").ap()

    xbf_in = din("xbf", (T, D), BF)
    xsb_in = din("xsb", (SH, D), F32)        # x shard with sa_bo folded in
    xaT_in = din("xaT", (128, KT * T), BF)   # packed (p)(k,t)
    w_ins = {}
    for nm in ("wq1", "wk1", "wv1", "wq2", "wk2", "wv2"):
        w_ins[nm] = din(nm, (128, KT * NQ), BF)       # packed (p)(k,c)
    w_ins["wo1"] = din("wo1", (NCH * 128, KO * 512), BF)
    w_ins["wo2"] = din("wo2", (NCH * 128, KO * 512), BF)
    w_ins["fc1w"] = din("fc1w", (KF * 128, KT * 128), BF)
    w_ins["fc2w"] = din("fc2w", (NCH * 128, KF * 512), BF)
    qb1_in = din("qb1", (128, HPC), F32)     # col h = head-h bias (80 rows used)
    kb1_in = din("kb1", (128, HPC), F32)
    qb2_in = din("qb2", (128, HPC), F32)
    kb2_in = din("kb2", (128, HPC), F32)
    vb1_in = din("vb1", (1, NQ), BF)
    vb2_in = din("vb2", (1, NQ), BF)
    fc1b_in = din("fc1b", (128, KF), F32)
    maskT_in = din("maskT", (128, T), BF)    # [k, b*128+q] 0/1 self-attn mask
    crossb_in = din("crossb", (128, B), F32)  # additive bias per (k, b)
    ident_in = din("ident", (128, 128), BF)
    bo2t_in = din("bo2t", (SH, D), BF)
    fb2t_in = din("fb2t", (SH, D), BF)

    kT1_out = dout("kT1", (NQ, T), BF)
    vt1_out = dout("vt1", (T, NQ), BF)
    kT2_out = dout("kT2", (NQ, T), BF)
    vt2_out = dout("vt2", (T, NQ), BF)
    x_out = dout("xout", (SH, D), F32)

    with tile.TileContext(nc) as tc:
        with tc.tile_pool(name="dram", bufs=1, space="DRAM") as dpool, \
             tc.tile_pool(name="psum", bufs=8, space="PSUM") as pspool, \
             tc.tile_pool(name="sb", bufs=1) as sb:

            counter = [0]

            def alloc(tag, shape, dtype, bufs):
                counter[0] += 1
                return sb.tile(shape, dtype, name=f"{tag}{counter[0]}",
                               tag=tag, bufs=bufs)

            def psum(shape=(128, 512), dtype=F32):
                counter[0] += 1
                return pspool.tile(list(shape), dtype, name=f"ps{counter[0]}",
                                   tag="ps")

            dma_in = nc.sync.dma_start      # loads on the SP HWDGE queue
            dma_out = nc.scalar.dma_start   # stores on the ACT HWDGE queue

            # ---------------- DRAM bounce buffers for collectives ----------
            rs1i = dpool.tile([T, D], BF, name="rs1i")
            rs1o = dpool.tile([SH, D], BF, name="rs1o")
            ag1i = dpool.tile([SH, D], BF, name="ag1i")
            ag1o = dpool.tile([T, D], BF, name="ag1o", addr_space="Shared")
            rs2i = dpool.tile([T, D], BF, name="rs2i")
            rs2o = dpool.tile([SH, D], BF, name="rs2o")
            ag2i = dpool.tile([SH, D], BF, name="ag2i")
            ag2o = dpool.tile([T, D], BF, name="ag2o", addr_space="Shared")
            rs3i = dpool.tile([T, D], BF, name="rs3i")
            rs3o = dpool.tile([SH, D], BF, name="rs3o")

            # ---------------- small constants (live whole kernel) ---------
            qb1_sb = sb.tile([128, HPC], F32, name="qb1_sb")
            kb1_sb = sb.tile([128, HPC], F32, name="kb1_sb")
            qb2_sb = sb.tile([128, HPC], F32, name="qb2_sb")
            kb2_sb = sb.tile([128, HPC], F32, name="kb2_sb")
            vb1_sb = sb.tile([1, NQ], BF, name="vb1_sb")
            vb2_sb = sb.tile([1, NQ], BF, name="vb2_sb")
            fc1b_sb = sb.tile([128, KF], F32, name="fc1b_sb")
            maskT_sb = sb.tile([128, T], BF, name="maskT_sb")
            crossb_sb = sb.tile([128, B], F32, name="crossb_sb")
            ident_sb = sb.tile([128, 128], BF, name="ident_sb")
            onesc_sb = sb.tile([128, 1], BF, name="onesc_sb")   # ones column
            onesr_sb = sb.tile([1, 128], BF, name="onesr_sb")   # ones row
            eps_sb = sb.tile([128, 1], F32, name="eps_sb")
            for t_, s_ in ((qb1_sb, qb1_in), (kb1_sb, kb1_in), (qb2_sb, qb2_in),
                           (kb2_sb, kb2_in), (vb1_sb, vb1_in), (vb2_sb, vb2_in),
                           (fc1b_sb, fc1b_in), (maskT_sb, maskT_in),
                           (crossb_sb, crossb_in), (ident_sb, ident_in)):
                dma_in(t_[:], s_[:])
            nc.vector.memset(onesc_sb[:], 1.0)
            nc.vector.memset(onesr_sb[:], 1.0)
            nc.vector.memset(eps_sb[:], EPS)

            # residual accumulator (in-place through the whole chain)
            xres = alloc("xres", [SH, D], F32, 1)
            dma_in(xres[:], xsb_in[:])

            # ---------------- helpers -------------------
            def pet(dst_ap, src_ap):
                # PE transpose: src [128,128] bf16 SBUF -> dst [128,128] SBUF
                tp = psum((128, 128), BF)
                nc.tensor.transpose(tp[:], src_ap, ident_sb[:])
                nc.any.tensor_copy(dst_ap, tp[:])

            def load_w320(nm):
                w = alloc("w320", [128, KT * NQ], BF, 3)
                dma_in(w[:], w_ins[nm][:])
                return w

            def layernorm(dst_bf, src, P):
                # src [P, D] -> dst [P, D] bf16 normalized (no gamma/beta)
                bnt = alloc("ln_bnt", [P, 5 * 6], F32, 4)
                mv = alloc("ln_mv", [P, 2], F32, 4)
                istd = alloc("ln_istd", [P, 1], F32, 4)
                nmu = alloc("ln_nmu", [P, 1], F32, 4)
                for c in range(5):
                    nc.vector.bn_stats(bnt[:, 6 * c:6 * (c + 1)],
                                       src[:, 512 * c:512 * (c + 1)])
                nc.vector.bn_aggr(mv[:], bnt[:])
                nc.scalar.activation(istd[:], mv[:, 1:2], AF.Sqrt,
                                     bias=eps_sb[0:P, :])
                nc.vector.reciprocal(istd[:], istd[:])
                nc.vector.tensor_mul(nmu[:], mv[:, 0:1], istd[:])
                nc.vector.tensor_scalar_mul(nmu[:], nmu[:], -1.0)
                nc.scalar.activation(dst_bf, src, AF.Identity,
                                     bias=nmu[:], scale=istd[:])

            # ---------------- projections ----------------
            def proj_qk(w, bias_sb, hT, name):
                # per-head feature-major tiles [80, T]
                out_tiles = []
                for h in range(HPC):
                    ps = psum()
                    for k in range(KT):
                        nc.tensor.matmul(
                            ps[0:80, :],
                            w[:, NQ * k + 80 * h:NQ * k + 80 * h + 80],
                            hT(k), start=(k == 0), stop=(k == KT - 1))
                    o = alloc("qkT", [80, T], BF, 12)
                    nc.scalar.activation(o[:], ps[0:80, :], AF.Identity,
                                         bias=bias_sb[0:80, h:h + 1])
                    out_tiles.append(o)
                return out_tiles

            def proj_v(w, bias_sb, hT, name):
                # token-major [128, NQ] tiles; bias added as a rank-1 matmul
                out_tiles = []
                for mt in range(NTOK):
                    ps = psum((128, NQ))
                    nc.tensor.matmul(ps[:], onesr_sb[:], bias_sb[:],
                                     start=True, stop=False)
                    for k in range(KT):
                        nc.tensor.matmul(
                            ps[:], hT(k)[:, 128 * mt:128 * (mt + 1)],
                            w[:, NQ * k:NQ * (k + 1)],
                            start=False, stop=(k == KT - 1))
                    o = alloc("vt", [128, NQ], BF, 8)
                    nc.scalar.copy(o[:], ps[:])
                    out_tiles.append(o)
                return out_tiles

            # ---------------- attention ----------------
            def attention(qT, kT, vtok, cross, name):
                # token-major o_cat [128, NQP] per batch (per-partition 1/Z),
                # then PE-transposed into packed feature-major tiles.
                ocatT = [alloc("ocatT", [128, T], BF, 6) for _ in range(KO)]
                for b in range(B):
                    cs = slice(128 * b, 128 * (b + 1))
                    oc = alloc("at_oc", [128, NQP], BF, 2)
                    nc.vector.memset(oc[:, NQ:NQP], 0.0)
                    for h in range(HPC):
                        sps = psum((128, 128))
                        nc.tensor.matmul(sps[:], kT[h][:, cs], qT[h][:, cs],
                                         start=True, stop=True)
                        p = alloc("at_p", [128, 128], BF, 4)
                        if cross:
                            nc.scalar.activation(p[:], sps[:], AF.Exp,
                                                 bias=crossb_sb[:, b:b + 1])
                        else:
                            nc.scalar.activation(p[:], sps[:], AF.Exp)
                            nc.vector.tensor_mul(p[:], p[:], maskT_sb[:, cs])
                        ops = psum((128, 128))
                        nc.tensor.matmul(ops[:, 0:80], p[:],
                                         vtok[b][:, 80 * h:80 * h + 80],
                                         start=True, stop=True)
                        nc.tensor.matmul(ops[:, 80:81], p[:], onesc_sb[:],
                                         start=True, stop=True)
                        zinv = alloc("at_zi", [128, 1], F32, 4)
                        nc.vector.reciprocal(zinv[:], ops[:, 80:81])
                        nc.scalar.activation(oc[:, 80 * h:80 * h + 80],
                                             ops[:, 0:80], AF.Copy,
                                             scale=zinv[:])
                    for c in range(KO):
                        pet(ocatT[c][:, cs], oc[:, 128 * c:128 * (c + 1)])
                return ocatT

            # ---------------- output projection ----------------
            def out_proj(ocatT, wo_nm, dst_dram):
                for nch in range(NCH):
                    won = alloc("won", [128, KO * 512], BF, 2)
                    dma_in(won[:], w_ins[wo_nm][128 * nch:128 * (nch + 1), :])
                    for mt in range(NTOK):
                        ps = psum()
                        for k in range(KO):
                            nc.tensor.matmul(
                                ps[:], ocatT[k][:, 128 * mt:128 * (mt + 1)],
                                won[:, 512 * k:512 * (k + 1)],
                                start=(k == 0), stop=(k == KO - 1))
                        stage = alloc("stage", [128, 512], BF, 3)
                        nc.any.tensor_copy(stage[:], ps[:])
                        dma_out(
                            dst_dram[128 * mt:128 * (mt + 1),
                                     512 * nch:512 * (nch + 1)], stage[:])

            # ---------------- sharded residual + LN ----------------
            def resid_ln(rs_out_dram, bias_in, ag_in_dram, name, last=False):
                rssb = alloc("res16", [SH, D], BF, 2)
                dma_in(rssb[:], rs_out_dram[:])
                nc.vector.tensor_add(xres[:], xres[:], rssb[:])
                if bias_in is not None:
                    bt = alloc("biast", [SH, D], BF, 1)
                    dma_in(bt[:], bias_in[:])
                    nc.vector.tensor_add(xres[:], xres[:], bt[:])
                if last:
                    dma_out(x_out[:], xres[:])
                    return
                hsh = alloc("res16", [SH, D], BF, 2)
                layernorm(hsh[:], xres[:], SH)
                dma_out(ag_in_dram[:], hsh[:])

            def collective(kind, op, i, o):
                nc.gpsimd.collective_compute(kind, op, replica_groups=RG,
                                             ins=[i.opt()], outs=[o.opt()])

            def load_T(src_dram, name):
                # token-major DRAM [T, D] -> feature-major big tile via PE
                big = alloc("actT", [128, KT * T], BF, 2)
                for mt in range(NTOK):
                    ld = alloc("agld", [128, D], BF, 2)
                    dma_in(ld[:], src_dram[128 * mt:128 * (mt + 1), :])
                    for k in range(KT):
                        pet(big[:, T * k + 128 * mt:T * k + 128 * (mt + 1)],
                            ld[:, 128 * k:128 * (k + 1)])
                return lambda k: big[:, T * k:T * (k + 1)]

            # =================== phase 0: LN1 + transpose ==============
            wq1 = load_w320("wq1")
            wk1 = load_w320("wk1")
            wv1 = load_w320("wv1")
            xaTbig = alloc("actT", [128, KT * T], BF, 2)
            dma_in(xaTbig[:], xaT_in[:])
            xaT = lambda k: xaTbig[:, T * k:T * (k + 1)]  # noqa: E731
            h1Tbig = alloc("actT", [128, KT * T], BF, 2)
            h1T = lambda k: h1Tbig[:, T * k:T * (k + 1)]  # noqa: E731
            for mt in range(NTOK):
                xt = alloc("xt", [128, D], BF, 1)
                dma_in(xt[:], xbf_in[128 * mt:128 * (mt + 1), :])
                h1t = alloc("h1t", [128, D], BF, 2)
                layernorm(h1t[:], xt[:], 128)
                for k in range(KT):
                    pet(h1Tbig[:, T * k + 128 * mt:T * k + 128 * (mt + 1)],
                        h1t[:, 128 * k:128 * (k + 1)])

            # =================== self attention ===================
            qT1 = proj_qk(wq1, qb1_sb, h1T, "qT1")
            kT1 = proj_qk(wk1, kb1_sb, h1T, "kT1")
            vt1 = proj_v(wv1, vb1_sb, h1T, "vt1")
            for h in range(HPC):
                dma_out(kT1_out[80 * h:80 * h + 80, :], kT1[h][:])
            for mt in range(NTOK):
                dma_out(vt1_out[128 * mt:128 * (mt + 1), :], vt1[mt][:])
            oc1 = attention(qT1, kT1, vt1, cross=False, name="a1")
            out_proj(oc1, "wo1", rs1i)
            collective("ReduceScatter", ALU.add, rs1i, rs1o)

            # cross K/V (independent of the self-attention result)
            wk2 = load_w320("wk2")
            wv2 = load_w320("wv2")
            kT2 = proj_qk(wk2, kb2_sb, xaT, "kT2")
            vt2 = proj_v(wv2, vb2_sb, xaT, "vt2")
            for h in range(HPC):
                dma_out(kT2_out[80 * h:80 * h + 80, :], kT2[h][:])
            for mt in range(NTOK):
                dma_out(vt2_out[128 * mt:128 * (mt + 1), :], vt2[mt][:])

            resid_ln(rs1o, None, ag1i, "r1")
            collective("AllGather", ALU.bypass, ag1i, ag1o)
            h2T = load_T(ag1o, "h2T")

            # =================== cross attention ===================
            wq2 = load_w320("wq2")
            qT2 = proj_qk(wq2, qb2_sb, h2T, "qT2")
            oc2 = attention(qT2, kT2, vt2, cross=True, name="a2")
            out_proj(oc2, "wo2", rs2i)
            collective("ReduceScatter", ALU.add, rs2i, rs2o)
            resid_ln(rs2o, bo2t_in, ag2i, "r2")
            collective("AllGather", ALU.bypass, ag2i, ag2o)
            h3T = load_T(ag2o, "h3T")

            # =================== MLP ===================
            uT = []
            for m in range(KF):
                fm = alloc("fc1m", [128, KT * 128], BF, 2)
                dma_in(fm[:], w_ins["fc1w"][128 * m:128 * (m + 1), :])
                ps = psum()
                for k in range(KT):
                    nc.tensor.matmul(ps[:], fm[:, 128 * k:128 * (k + 1)],
                                     h3T(k), start=(k == 0),
                                     stop=(k == KT - 1))
                u = alloc("uT", [128, T], BF, 10)
                nc.scalar.activation(u[:], ps[:], AF.Gelu,
                                     bias=fc1b_sb[:, m:m + 1])
                uT.append(u)
            for nch in range(NCH):
                fn = alloc("fc2n", [128, KF * 512], BF, 2)
                dma_in(fn[:], w_ins["fc2w"][128 * nch:128 * (nch + 1), :])
                for mt in range(NTOK):
                    ps = psum()
                    for k in range(KF):
                        nc.tensor.matmul(
                            ps[:], uT[k][:, 128 * mt:128 * (mt + 1)],
                            fn[:, 512 * k:512 * (k + 1)],
                            start=(k == 0), stop=(k == KF - 1))
                    stage = alloc("stage", [128, 512], BF, 3)
                    nc.any.tensor_copy(stage[:], ps[:])
                    dma_out(rs3i[128 * mt:128 * (mt + 1),
                                 512 * nch:512 * (nch + 1)], stage[:])
            collective("ReduceScatter", ALU.add, rs3i, rs3o)
            resid_ln(rs3o, fb2t_in, None, "r3", last=True)

    nc.compile()
    return nc


def _pack_col(w):
    # [D, n] -> [128, KT*n] packed (p)(k, c)
    n = w.shape[1]
    return np.ascontiguousarray(
        w.reshape(KT, 128, n).transpose(1, 0, 2).reshape(128, KT * n))


def _prep_inputs(inputs):
    f32 = np.float32
    x = np.asarray(inputs["x"], f32).reshape(T, D)
    xa = np.asarray(inputs["xa"], f32).reshape(T, D)
    mask = np.asarray(inputs["mask"])
    cross_mask = np.asarray(inputs["cross_mask"])
    g1 = np.asarray(inputs["ln1_g"], f32)
    b1 = np.asarray(inputs["ln1_b"], f32)
    g2 = np.asarray(inputs["ln2_g"], f32)
    b2 = np.asarray(inputs["ln2_b"], f32)
    g3 = np.asarray(inputs["ln3_g"], f32)
    b3 = np.asarray(inputs["ln3_b"], f32)
    scale = f32(1.0 / math.sqrt(HD))

    xaT = np.ascontiguousarray(xa.T)                       # [D, T]
    xaT_pk = _pack_col(xaT.reshape(D, T)).astype(BF16)

    def colshard(w, b, g, bfold, r, n, sc=1.0):
        w = np.asarray(w, f32)
        b = np.asarray(b, f32)
        cols = slice(n * r, n * (r + 1))
        weff = w[:, cols] * (g[:, None] * f32(sc))
        beff = (bfold @ w[:, cols] + b[cols]) * f32(sc)
        return _pack_col(weff).astype(BF16), beff

    def headbias(beff):
        out = np.zeros((HPC, 128), f32)
        out[:, :HD] = beff.reshape(HPC, HD)
        return np.ascontiguousarray(out.T)

    def rowblocks(w, r, nrows):
        # [nrows, D] row shard -> padded [NQP, D] -> [NCH*128, KO*512]
        w = np.asarray(w, f32)[nrows * r:nrows * (r + 1), :]
        wp = np.zeros((NQP, D), f32)
        wp[:nrows] = w
        blk = wp.reshape(KO, 128, NCH, 512).transpose(2, 1, 0, 3)
        return np.ascontiguousarray(blk.reshape(NCH * 128, KO * 512)).astype(BF16)

    # self-attn mask -> multiplicative, transposed to [k, b*128+q]
    m01 = mask[:, 0, :, :S].astype(f32)                      # [B, q, k]
    maskT = np.ascontiguousarray(
        m01.transpose(2, 0, 1).reshape(S, B * S)).astype(BF16)
    cb = np.where(cross_mask[:, 0, 0, :S], f32(0), f32(-1e9))  # [B, k]
    crossb = np.ascontiguousarray(cb.T)                        # [k, B]
    ident = np.eye(128, dtype=f32).astype(BF16)

    ones_g = np.ones_like(g1)
    zb = np.zeros_like(b1)
    in_maps = []
    for r in range(NCORES):
        wq1, qb1 = colshard(inputs["sa_wq"], inputs["sa_bq"], g1, b1, r, NQ,
                            scale)
        wk1, kb1 = colshard(inputs["sa_wk"], inputs["sa_bk"], g1, b1, r, NQ)
        wv1, vb1 = colshard(inputs["sa_wv"], inputs["sa_bv"], g1, b1, r, NQ)
        wq2, qb2 = colshard(inputs["ca_wq"], inputs["ca_bq"], g2, b2, r, NQ,
                            scale)
        # cross K/V read raw xa: no LN fold
        wk2, kb2 = colshard(inputs["ca_wk"], inputs["ca_bk"], ones_g, zb, r, NQ)
        wv2, vb2 = colshard(inputs["ca_wv"], inputs["ca_bv"], ones_g, zb, r, NQ)
        fc1, fc1b = colshard(inputs["fc1_w"], inputs["fc1_b"], g3, b3, r, FFNS)
        # repack fc1 from (p)(k,c) to [KF*128 rows, KT*128 cols]
        fc1full = np.asarray(inputs["fc1_w"], f32)[:, FFNS * r:FFNS * (r + 1)] \
            * g3[:, None]
        fc1blk = fc1full.reshape(KT, 128, KF, 128).transpose(2, 1, 0, 3)
        fc1blk = fc1blk.reshape(KF * 128, KT * 128)
        fc2 = np.asarray(inputs["fc2_w"], f32)[FFNS * r:FFNS * (r + 1), :]
        fc2blk = fc2.reshape(KF, 128, NCH, 512).transpose(2, 1, 0, 3)
        fc2blk = fc2blk.reshape(NCH * 128, KF * 512)
        xsb = x[SH * r:SH * (r + 1), :] + np.asarray(inputs["sa_bo"], f32)[None, :]
        in_maps.append({
            "xbf": x.astype(BF16), "xsb": np.ascontiguousarray(xsb),
            "xaT": xaT_pk,
            "wq1": wq1, "wk1": wk1, "wv1": wv1,
            "wq2": wq2, "wk2": wk2, "wv2": wv2,
            "wo1": rowblocks(inputs["sa_wo"], r, NQ),
            "wo2": rowblocks(inputs["ca_wo"], r, NQ),
            "fc1w": np.ascontiguousarray(fc1blk).astype(BF16),
            "fc2w": np.ascontiguousarray(fc2blk).astype(BF16),
            "qb1": headbias(qb1), "kb1": headbias(kb1),
            "qb2": headbias(qb2), "kb2": headbias(kb2),
            "vb1": vb1[None, :].astype(BF16), "vb2": vb2[None, :].astype(BF16),
            "fc1b": np.ascontiguousarray(fc1b.reshape(KF, 128).T),
            "maskT": maskT, "crossb": crossb, "ident": ident,
            "bo2t": np.broadcast_to(np.asarray(inputs["ca_bo"], f32),
                                    (SH, D)).astype(BF16),
            "fb2t": np.broadcast_to(np.asarray(inputs["fc2_b"], f32),
                                    (SH, D)).astype(BF16),
        })
    return in_maps


def _gather(inputs, results):
    f32 = np.float32
    x_out = np.concatenate([results[r]["xout"] for r in range(NCORES)],
                           axis=0).reshape(B, S, D).astype(f32)

    def cache_fill(cache_in, key):
        out = np.array(cache_in, dtype=f32)
        for r in range(NCORES):
            arr = np.asarray(results[r][key], dtype=f32)
            if key.startswith("kT"):
                arr = arr.T                     # -> [T, NQ]
            blk = arr.reshape(B, S, HPC, HD).transpose(0, 2, 1, 3)
            out[:, HPC * r:HPC * (r + 1), :S, :] = blk
        return out

    sk = cache_fill(inputs["cache_sk"], "kT1")
    sv = cache_fill(inputs["cache_sv"], "vt1")
    ck = cache_fill(inputs["cache_ck"], "kT2")
    cv = cache_fill(inputs["cache_cv"], "vt2")
    return x_out, sk, sv, ck, cv


def _run(in_maps, trace=False):
    from concourse.bass_utils import run_bass_kernel_spmd
    return run_bass_kernel_spmd(_STATE["nc"], in_maps,
                                core_ids=list(range(NCORES)), trace=trace)


def kernel(**inputs):
    if "nc" not in _STATE:
        _STATE["nc"] = _build()
    in_maps = _prep_inputs(inputs)
    res = _run(in_maps)
    return _gather(inputs, res.results)
